# revision 1
# baseline (speedup 1.0000x reference)
"""Biased axial attention (RoseTTAFold-style) on 8 TRN2 NeuronCores.

nn_BiasedAxialAttention: O=1, L=384, d_pair=d_bias=128, H=4, DH=32.

  p    = LN(pair^T);  bsrc = LN(bias^T)            (LN over d per position)
  q,k,v,gate projections of p; b = bsrc @ Wb^T
  attn[i,j,h] = sum_{n,k} q[n,i,h,k] k[n,j,h,k] + b[i,j,h]
  A = softmax_j(attn);  out[n,i,:] = (gate * einsum(A, v)) @ Wo^T + bo
  result[i,n,:] = out[n,i,:]

Sharding: the non-attended axial dim n, 48 rows/core. The logit contraction
spans ALL n, so each core computes partial logits Z_c[i,j,h] for its n-slice
and an on-chip AllReduce sums them; the bias term B (each core computes its
i-slice) is shared via a small AllGather. Everything else is local. The host
only slices, permutes and concatenates.

Per-core dataflow (validated against the reference in numpy, see proto.py):
  1. stream pair slice row-major: bn_stats LN -> fused normalize+bf16 cast
     (ACT, per-partition scale/bias) -> PE transpose -> XT [d,(n,i)]
  2. same for bias slice -> B projection [4h, j] per i-row -> DRAM -> AllGather
  3. Q/K projections [hk,(n,i)], DMA-shuffled to [(4n,32k), i] tiles per
     (head, n-group); V/G projected after the logit matmuls (they overlap the
     AllReduce): V "flipped" (X-block stationary) into [j,(n,hd)], G standard
     with fused sigmoid(+bg)
  4. logits: per (head, i-block) 12 accumulating 128x128x384 matmuls -> Z
  5. Z -> DRAM -> AllReduce -> Z += B -> softmax_j (max/exp/recip, fused
     accumulation) -> A -> PE transpose per head -> AT [j, i]
  6. A@V: per (head, 4n-group) psum [(4n,32d), i]; evicted fused with the
     gate multiply (partition-shifted tensor_tensor) -> Gated [hd,(n,i)]
  7. output projection flipped (Gated-block stationary) -> row-major [+bo]
     -> DRAM [18432, 128]
"""
import sys

if "/opt/trn_rl_repo" not in sys.path:
    sys.path.insert(0, "/opt/trn_rl_repo")

import numpy as np
from contextlib import ExitStack

import concourse.bass as bass
import concourse.bacc as bacc
import concourse.mybir as mybir
import concourse.tile as tile
from concourse.bass_utils import run_bass_kernel_spmd
from concourse.masks import make_identity

F32 = mybir.dt.float32
BF16 = mybir.dt.bfloat16
AF = mybir.ActivationFunctionType
ALU = mybir.AluOpType
AX = mybir.AxisListType

O, L, DP, H, DH = 1, 384, 128, 4, 32
HD = H * DH
NCORES = 8
NS = L // NCORES            # 48 n's per core
R = NS * L                  # 18432 rows per core
NBLK = R // 128             # 144 row-blocks
NG = NS // 4                # 12 logit contraction groups (4 n's each)
IB = L // 128               # 3 blocks of 128 along i/j
SCALING = 1.0 / np.sqrt(DH)
EPS = 1e-5

CH = 6                      # LN streaming chunk: 6 row-blocks = 768 rows = 2 i-rows
NCHUNK = NBLK // CH         # 24


def build_tile_kernel(ctx: ExitStack, tc: tile.TileContext, outs, ins):
    """Emit the per-core program. outs/ins: dicts of bass.AP keyed by name."""
    nc = tc.nc
    pairc = ins["pairc"].rearrange("(b p) d -> p b d", p=128)   # [128,144,128]
    biasc = ins["biasc"].rearrange("(b p) d -> p b d", p=128)
    outc = outs["outc"].rearrange("(b p) d -> p b d", p=128)

    const = ctx.enter_context(tc.tile_pool(name="const", bufs=1))
    big = ctx.enter_context(tc.tile_pool(name="big", bufs=1))
    stream = ctx.enter_context(tc.tile_pool(name="stream", bufs=2))
    lnload = ctx.enter_context(tc.tile_pool(name="lnload", bufs=6))
    outstg = ctx.enter_context(tc.tile_pool(name="outstg", bufs=3))
    evqp = ctx.enter_context(tc.tile_pool(name="evqp", bufs=3))
    ps_acc = ctx.enter_context(tc.tile_pool(name="psacc", bufs=2, space="PSUM"))
    ps_tr = ctx.enter_context(tc.tile_pool(name="pstr", bufs=3, space="PSUM"))
    ps_pj = ctx.enter_context(tc.tile_pool(name="pspj", bufs=1, space="PSUM"))
    dram = ctx.enter_context(tc.tile_pool(name="dram", bufs=1, space="DRAM"))

    # ---------------- stage 0: constants / weights -------------------------
    ident = const.tile([128, 128], BF16)
    make_identity(nc, ident)
    epst = const.tile([128, 1], F32)
    nc.vector.memset(epst[:], EPS)

    vecs = const.tile([128, 4], F32)          # cols: gp, gb, bg, bo
    nc.sync.dma_start(vecs[:], ins["vecs"][:])
    borow = const.tile([1, 128], F32)
    nc.sync.dma_start(borow[:], ins["borow"][:])
    borow_b = const.tile([1, 128], BF16)
    nc.vector.tensor_copy(borow_b[:], borow[:])

    wf = {}
    for nm in ("wqt", "wkt", "wvt", "wgt", "wot"):
        t = const.tile([128, 128], F32, name=f"{nm}_f")
        nc.sync.dma_start(t[:], ins[nm][:])
        wf[nm] = t
    wbt_f = const.tile([128, 4], F32)
    nc.sync.dma_start(wbt_f[:], ins["wbt"][:])

    # fold LN gain + scale constants into bf16 lhsT weights (per-partition d)
    wb = {}
    for nm, extra in (("wqt", SCALING), ("wkt", 1.0 / L), ("wvt", 1.0),
                      ("wgt", 1.0)):
        gs = const.tile([128, 1], F32, name=f"{nm}_gs")
        nc.vector.tensor_scalar_mul(gs[:], vecs[:, 0:1], extra)
        t = const.tile([128, 128], BF16, name=f"{nm}_b")
        nc.vector.tensor_scalar_mul(t[:], wf[nm][:], gs[:, 0:1])
        wb[nm] = t
    wot_b = const.tile([128, 128], BF16)
    nc.vector.tensor_copy(wot_b[:], wf["wot"][:])
    wbt_b = const.tile([128, 4], BF16)
    nc.vector.tensor_scalar_mul(wbt_b[:], wbt_f[:], vecs[:, 1:2])

    # ---------------- persistent SBUF / DRAM tensors -----------------------
    xt = big.tile([128, NS, L], BF16, tag="xt")        # X^T [d,(n,i)]
    qs = big.tile([128, H, NG, L], BF16, tag="qs")     # [(4n,32k), h, g, i]
    ks = big.tile([128, H, NG, L], BF16, tag="ks")
    # bf16: the AllReduce carries only the tiny q.k partial sums (~3e-3 std);
    # the dominant bias term is added post-AR from the f32 AllGather
    zsb = big.tile([128, IB, H, L], BF16, tag="z")     # logits [i%128, ib, h, j]
    asb = big.tile([128, IB, H, L], BF16, tag="a")     # A      [i%128, ib, h, j]

    bgin = dram.tile([NS, H, L], F32)                  # local B [il, h, j]
    bgout = dram.tile([L, H, L], F32, addr_space="Shared")
    zin = dram.tile([128, IB * H * L], BF16)
    zout = dram.tile([128, IB * H * L], BF16, addr_space="Shared")

    # ---------------- stage 1: LayerNorm + transpose -----------------------
    # Normalization is folded into the PE transpose: X^T-block = rm^T @
    # diag(r) (per-row scale) plus a K=1 rank-1 matmul adding -mu*r per row.
    # The input is cast f32->bf16 in-flight by the SWDGE load.
    onesrow = const.tile([1, 128], BF16)
    nc.vector.memset(onesrow[:], 1.0)

    def ln_chunk(src_dram, ch, dst_cb):
        rm = lnload.tile([128, CH, 128], BF16, tag="lnrm")
        nc.gpsimd.dma_start(rm[:], src_dram[:, ch * CH:(ch + 1) * CH, :])
        st = stream.tile([128, CH, 6], F32, tag="lnst")
        for b in range(CH):
            nc.vector.bn_stats(st[:, b, :], rm[:, b, :])
        v0 = stream.tile([128, CH], F32, tag="lnv0")
        dm = stream.tile([128, CH], F32, tag="lndm")
        r = stream.tile([128, CH], F32, tag="lnr")
        nmr = stream.tile([128, CH], F32, tag="lnnmr")
        # var = (cv_e+cv_o)/128 + ((m_e-m_o)/2)^2 ; mean = (m_e+m_o)/2
        nc.vector.tensor_tensor(v0[:], st[:, :, 2], st[:, :, 5], ALU.add)
        nc.vector.tensor_scalar_mul(v0[:], v0[:], 1.0 / 128)
        nc.vector.tensor_tensor(dm[:], st[:, :, 1], st[:, :, 4], ALU.subtract)
        nc.vector.tensor_tensor(dm[:], dm[:], dm[:], ALU.mult)
        nc.vector.tensor_scalar_mul(dm[:], dm[:], 0.25)
        nc.vector.tensor_tensor(v0[:], v0[:], dm[:], ALU.add)
        nc.scalar.activation(r[:], v0[:], AF.Sqrt, bias=epst[:, 0:1])
        nc.vector.reciprocal(r[:], r[:])
        nc.vector.tensor_tensor(nmr[:], st[:, :, 1], st[:, :, 4], ALU.add)
        nc.vector.tensor_tensor(nmr[:], nmr[:], r[:], ALU.mult)
        nc.vector.tensor_scalar_mul(nmr[:], nmr[:], -0.5)
        norm = stream.tile([128, CH, 128], BF16, tag="lnnorm")
        for b in range(CH):
            if b % 2 == 0:
                nc.scalar.activation(norm[:, b, :], rm[:, b, :], AF.Identity,
                                     bias=nmr[:, b:b + 1], scale=r[:, b:b + 1])
            else:
                nc.vector.tensor_scalar(norm[:, b, :], rm[:, b, :],
                                        r[:, b:b + 1], nmr[:, b:b + 1],
                                        ALU.mult, ALU.add)
        for b in range(CH):
            pt = ps_tr.tile([128, 128], BF16, tag="tr")
            nc.tensor.transpose(pt[:], norm[:, b, :], ident[:])
            dst_cb(ch * CH + b, pt)

    def pair_dst(bg, pt):
        n, isub = bg // IB, bg % IB
        dst = xt[:, n, isub * 128:(isub + 1) * 128]
        if bg % 2 == 0:
            nc.vector.tensor_copy(dst, pt[:])
        else:
            nc.scalar.copy(dst, pt[:])

    # ---------------- stages 1+3 interleaved -------------------------------
    # LN chunks 2g, 2g+1 cover exactly the 4 n's of Q/K group g, so emit each
    # group's projections right after its chunks: PE streams transposes and
    # projections back-to-back and the logits accumulation starts early.
    # qs/ks partition order within a group is (k-outer, nn-inner): p = 4k+nn.
    # The logit contraction is a dot product over partitions, so any order
    # works as long as qs and ks share it. This order lets one DMA per
    # (head, group) restack a 4-n staging tile [32k, 4nn, L] into the full
    # 128-partition tile [128, L] (walk orders match).
    for g in range(NG):
        ln_chunk(pairc, 2 * g, pair_dst)
        ln_chunk(pairc, 2 * g + 1, pair_dst)
        evq = {}
        for wname in ("wqt", "wkt"):
            evq[wname] = evqp.tile([128, 4, L], BF16, tag=f"evq{wname}",
                                   name=f"evq_{wname}_{g}")
        for nn in range(4):
            n = 4 * g + nn
            for idx, wname in enumerate(("wqt", "wkt")):
                pp = ps_acc.tile([128, L], F32, tag="acc")
                nc.tensor.matmul(pp[:], wb[wname][:], xt[:, n, :], start=True,
                                 stop=True)
                if (nn + idx) % 2 == 0:
                    nc.vector.tensor_copy(evq[wname][:, nn, :], pp[:])
                else:
                    nc.scalar.copy(evq[wname][:, nn, :], pp[:])
        for h in range(H):
            for wname, dst in (("wqt", qs), ("wkt", ks)):
                nc.sync.dma_start(dst[:, h, g, :],
                                  evq[wname][32 * h:32 * h + 32, :, :])

    # ---------------- stage 4: logits -------------------------------------
    for h in range(H):
        for ib in range(IB):
            lp = ps_acc.tile([128, L], F32, tag="acc")
            for g in range(NG):
                nc.tensor.matmul(lp[:], qs[:, h, g, ib * 128:(ib + 1) * 128],
                                 ks[:, h, g, :], start=(g == 0),
                                 stop=(g == NG - 1))
            nc.vector.tensor_copy(zsb[:, ib, h, :], lp[:])

    zsb_flat = zsb[:].rearrange("p a b c -> p (a b c)")
    nc.sync.dma_start(zin[:], zsb_flat)
    nc.gpsimd.collective_compute(
        "AllReduce", ALU.add, replica_groups=[list(range(NCORES))],
        ins=[zin[:].opt()], outs=[zout[:].opt()])

    # ---------------- stage 2 (placed here to overlap the AllReduce): ------
    # bias LN -> transient feature-major chunk -> B projection -> AllGather
    for ch in range(NCHUNK):
        btc = stream.tile([128, CH, 128], BF16, tag="btc")

        def bias_dst(bg, pt, btc=btc):
            bl = bg % CH
            if bg % 2 == 0:
                nc.vector.tensor_copy(btc[:, bl, :], pt[:])
            else:
                nc.scalar.copy(btc[:, bl, :], pt[:])

        ln_chunk(biasc, ch, bias_dst)
        bst = stream.tile([4, CH // 3, L], F32, tag="bstg")
        for il in range(CH // 3):
            bp = ps_pj.tile([4, L], F32, tag="pj4")
            nc.tensor.matmul(bp[:], wbt_b[:], btc[:, 3 * il:3 * il + 3, :],
                             start=True, stop=True)
            nc.scalar.copy(bst[:, il, :], bp[:])
        ilbase = ch * (CH // 3)
        nc.sync.dma_start(
            bgin[ilbase:ilbase + CH // 3, :, :].transpose([1, 0, 2]), bst[:])

    nc.gpsimd.collective_compute(
        "AllGather", ALU.bypass, replica_groups=[list(range(NCORES))],
        ins=[bgin[:].opt()], outs=[bgout[:].opt()])

    # ---------------- stage 6: G + V projections (overlap AR) --------------
    # G first: the gate tensor is consumed by the very first einsum group's
    # evicts, while vt's first consumer comes after softmax+A^T
    gsb = big.tile([128, NS, L], BF16, tag="ks")       # reuses ks slot
    for n in range(NS):
        gp_ = ps_acc.tile([128, L], F32, tag="acc")
        nc.tensor.matmul(gp_[:], wb["wgt"][:], xt[:, n, :], start=True,
                         stop=True)
        nc.scalar.activation(gsb[:, n, :], gp_[:], AF.Sigmoid,
                             bias=vecs[:, 2:3])

    # vt layout: [j%128, jb, h, g, nn, d] so the einsum's stationary slice
    # vt[:, jb, h, g, :, :] is one contiguous 128-wide free dim (walrus
    # requires a single free dim on the stationary operand)
    vt = big.tile([128, IB, H, NG, 4, DH], BF16, tag="qs")   # reuses qs slot
    for n in range(NS):
        for jb in range(IB):
            vp = ps_tr.tile([128, 128], F32, tag="tr")
            nc.tensor.matmul(vp[:], xt[:, n, jb * 128:(jb + 1) * 128],
                             wb["wvt"][:], start=True, stop=True)
            vdst = vt[:, jb, :, n // 4, n % 4, :]
            vsrc = vp[:].rearrange("p (h d) -> p h d", h=H)
            if (n + jb) % 2 == 0:
                nc.vector.tensor_copy(vdst, vsrc)
            else:
                nc.scalar.copy(vdst, vsrc)

    # ---------------- stage 5b: AR out + B add + softmax -------------------
    nc.sync.dma_start(zsb_flat, zout[:])
    badd = big.tile([128, IB, H, L], BF16, tag="at")
    nc.gpsimd.dma_start(badd[:], bgout[:].rearrange("(ib p) h l -> p ib h l",
                                                  p=128))
    badd_flat = badd[:].rearrange("p a b c -> p (a b c)")
    nc.vector.tensor_tensor(zsb_flat, zsb_flat, badd_flat, ALU.add)

    sums = stream.tile([128, IB, H], F32, tag="smsum")
    nmx = stream.tile([128, IB, H], F32, tag="smmax")
    for ib in range(IB):
        for h in range(H):
            nc.vector.tensor_reduce(nmx[:, ib, h:h + 1], zsb[:, ib, h, :],
                                    AX.X, ALU.max, negate=True)
            nc.scalar.activation(asb[:, ib, h, :], zsb[:, ib, h, :], AF.Exp,
                                 bias=nmx[:, ib, h:h + 1],
                                 accum_out=sums[:, ib, h:h + 1])
    rec = stream.tile([128, IB, H], F32, tag="smrec")
    nc.vector.reciprocal(rec[:], sums[:])
    for ib in range(IB):
        for h in range(H):
            nc.vector.tensor_scalar_mul(asb[:, ib, h, :], asb[:, ib, h, :],
                                        rec[:, ib, h:h + 1])

    # ---------------- stage 8: A^T per head --------------------------------
    at = big.tile([128, H, IB, L], BF16, tag="at")     # [j%128, h, jb, i]
    for h in range(H):
        for ib in range(IB):
            for jb in range(IB):
                ap_ = ps_tr.tile([128, 128], BF16, tag="tr")
                nc.tensor.transpose(
                    ap_[:], asb[:, ib, h, jb * 128:(jb + 1) * 128], ident[:])
                nc.vector.tensor_copy(
                    at[:, h, jb, ib * 128:(ib + 1) * 128], ap_[:])

    # ---------------- stages 9+10 interleaved ------------------------------
    # einsum A@V with fused gate (gq outer, heads inner), then immediately
    # the output projection + store for the 8 n's of that gq -- pipelines the
    # tail across PE / DVE / Sync instead of running three serial phases.
    gated = big.tile([128, NS, L], BF16, tag="xt")     # reuses xt slot
    gated_r = gated[:].rearrange("p (a b c) l -> p a b c l", b=2, c=4)
    gsb_r = gsb[:].rearrange("p (a b c) l -> p a b c l", b=2, c=4)
    gated_flat = gated[:].rearrange("p n l -> p (n l)")
    FB = 4
    for gq in range(NG // 2):
        for h in range(H):
            # 512-wide slots so each group's matmul stays inside one psum bank
            ep = ps_acc.tile([128, 2, 512], F32, tag="acc")
            for gg in range(2):
                g = 2 * gq + gg
                for jb in range(IB):
                    nc.tensor.matmul(
                        ep[:, gg, 0:L],
                        vt[:, jb, h, g, :, :].rearrange("p a b -> p (a b)"),
                        at[:, h, jb, :], start=(jb == 0), stop=(jb == IB - 1))
            # n = 4*(2gq+gg)+nn = 8gq + 4gg + nn ; plain evict copies split
            # ACT/DVE, gate applied full-width afterwards
            for nn in range(4):
                hb = slice(32 * h, 32 * h + 32)
                if (h + nn) % 2 == 0:
                    nc.vector.tensor_copy(gated_r[hb, gq, :, nn, :],
                                          ep[32 * nn:32 * nn + 32, :, 0:L])
                else:
                    nc.scalar.copy(gated_r[hb, gq, :, nn, :],
                                   ep[32 * nn:32 * nn + 32, :, 0:L])
        # full-width in-place gate for this gq's 8 n's
        nc.vector.tensor_tensor(
            gated[:, 8 * gq:8 * gq + 8, :], gated[:, 8 * gq:8 * gq + 8, :],
            gsb[:, 8 * gq:8 * gq + 8, :], ALU.mult)
        # output projection for rows of n in [8gq, 8gq+8) -> 24 row-blocks
        for fb in range(6):
            fst = outstg.tile([128, FB, 128], F32, tag="fstg")
            for q in range(FB):
                rb = gq * 24 + fb * FB + q
                fp = ps_tr.tile([128, 128], F32, tag="tr")
                nc.tensor.matmul(fp[:], gated_flat[:, rb * 128:(rb + 1) * 128],
                                 wot_b[:], start=True, stop=False)
                # bo added as a rank-1 K=1 matmul: ones^T x bo_row
                nc.tensor.matmul(fp[:], onesrow[:], borow_b[:], start=False,
                                 stop=True)
                if rb % 2 == 0:
                    nc.vector.tensor_copy(fst[:, q, :], fp[:])
                else:
                    nc.scalar.copy(fst[:, q, :], fp[:])
            fb_g = gq * 6 + fb
            nc.sync.dma_start(outc[:, fb_g * FB:(fb_g + 1) * FB, :], fst[:])


# ---------------------------------------------------------------------------
_NC_CACHE = {}


def _build_program():
    if "nc" in _NC_CACHE:
        return _NC_CACHE["nc"]
    nc = bacc.Bacc("TRN2", target_bir_lowering=False, debug=False,
                   enable_asserts=False, num_devices=NCORES)
    ins = {
        "pairc": nc.dram_tensor("pairc", [R, DP], F32, kind="ExternalInput").ap(),
        "biasc": nc.dram_tensor("biasc", [R, DP], F32, kind="ExternalInput").ap(),
        "wqt": nc.dram_tensor("wqt", [DP, HD], F32, kind="ExternalInput").ap(),
        "wkt": nc.dram_tensor("wkt", [DP, HD], F32, kind="ExternalInput").ap(),
        "wvt": nc.dram_tensor("wvt", [DP, HD], F32, kind="ExternalInput").ap(),
        "wgt": nc.dram_tensor("wgt", [DP, HD], F32, kind="ExternalInput").ap(),
        "wot": nc.dram_tensor("wot", [HD, DP], F32, kind="ExternalInput").ap(),
        "wbt": nc.dram_tensor("wbt", [DP, H], F32, kind="ExternalInput").ap(),
        "vecs": nc.dram_tensor("vecs", [DP, 4], F32, kind="ExternalInput").ap(),
        "borow": nc.dram_tensor("borow", [1, DP], F32, kind="ExternalInput").ap(),
    }
    outs = {
        "outc": nc.dram_tensor("outc", [R, DP], F32, kind="ExternalOutput").ap(),
    }
    with tile.TileContext(nc) as tc:
        with ExitStack() as ctx:
            build_tile_kernel(ctx, tc, outs, ins)
    nc.compile()
    _NC_CACHE["nc"] = nc
    return nc


def shard_inputs(pair, bias, ln_pair_g, ln_pair_b, ln_bias_g, ln_bias_b,
                 Wq, Wk, Wv, Wb, Wg, bg, Wo, bo):
    """Host-side slicing/permutation -> per-core input maps."""
    assert pair.shape == (O, L, L, DP) and bias.shape == (O, L, L, DP)
    assert np.abs(ln_pair_b).max() == 0 and np.abs(ln_bias_b).max() == 0, \
        "kernel folds LN beta=0; nonzero beta not implemented"
    f32 = np.float32
    shared = {
        "wqt": np.ascontiguousarray(Wq.T, f32),
        "wkt": np.ascontiguousarray(Wk.T, f32),
        "wvt": np.ascontiguousarray(Wv.T, f32),
        "wgt": np.ascontiguousarray(Wg.T, f32),
        "wot": np.ascontiguousarray(Wo.T, f32),
        "wbt": np.ascontiguousarray(Wb.T, f32),
        "vecs": np.ascontiguousarray(
            np.stack([ln_pair_g, ln_bias_g, bg, bo], axis=1), f32),
        "borow": np.ascontiguousarray(bo[None, :], f32),
    }
    in_maps = []
    for c in range(NCORES):
        S = slice(c * NS, (c + 1) * NS)
        m = dict(shared)
        m["pairc"] = np.ascontiguousarray(
            pair[0][:, S, :].transpose(1, 0, 2).reshape(R, DP), f32)
        m["biasc"] = np.ascontiguousarray(
            bias[0][:, S, :].transpose(1, 0, 2).reshape(R, DP), f32)
        in_maps.append(m)
    return in_maps


def gather_outputs(results):
    res = np.zeros((O, L, L, DP), np.float32)
    for c in range(NCORES):
        F = results[c]["outc"].reshape(NS, L, DP)
        res[0, :, c * NS:(c + 1) * NS, :] = F.transpose(1, 0, 2)
    return res


def kernel(**inputs):
    inputs = {k: np.asarray(v) for k, v in inputs.items()}
    nc = _build_program()
    in_maps = shard_inputs(**inputs)
    r = run_bass_kernel_spmd(nc, in_maps, core_ids=list(range(NCORES)))
    return gather_outputs(r.results)


def _ensure_ntff_hook():
    """The agent image's antenv lacks axon_hooks; recreate the registry and
    wire the ctypes NTFF hook from trn_agent_boot (profiling-only path)."""
    try:
        from antenv.axon_hooks import get_axon_ntff_profile_hook  # noqa: F401
        return
    except ImportError:
        pass
    import types
    import antenv
    mod = types.ModuleType("antenv.axon_hooks")
    mod._hook = None
    mod.set_axon_ntff_profile_hook = lambda h: setattr(mod, "_hook", h)
    mod.get_axon_ntff_profile_hook = lambda: mod._hook
    sys.modules["antenv.axon_hooks"] = mod
    antenv.axon_hooks = mod
    try:
        from trn_agent_boot.trn_boot import _ntff_profile_via_ctypes
        hook = _ntff_profile_via_ctypes("/opt/axon/libaxon_pjrt.so")
        if hook is not None:
            mod._hook = hook
    except Exception as e:  # profiling degrades, run still works
        print(f"NTFF hook setup failed: {e}", file=sys.stderr)


def kernel_profiled(**inputs):
    """Like kernel() but also returns exec-time info from neuron-profile."""
    inputs = {k: np.asarray(v) for k, v in inputs.items()}
    _ensure_ntff_hook()
    import concourse.bass_utils as bu
    bu.upload_artifacts = lambda tmpdir: f"local:{tmpdir}"  # no bucket here
    nc = _build_program()
    in_maps = shard_inputs(**inputs)
    r = run_bass_kernel_spmd(nc, in_maps, core_ids=list(range(NCORES)),
                             trace=True, trace_cores=list(range(NCORES)))
    return gather_outputs(r.results), r



# revision 8
# speedup vs baseline: 1.3173x; 1.3173x over previous
"""Biased axial attention (RoseTTAFold-style) on 8 TRN2 NeuronCores — v2.

nn_BiasedAxialAttention: O=1, L=384, d_pair=d_bias=128, H=4, DH=32.

  p    = LN(pair^T);  bsrc = LN(bias^T)            (LN over d per position)
  q,k,v,gate projections of p; b = bsrc @ Wb^T
  attn[i,j,h] = sum_{n,k} q[n,i,h,k] k[n,j,h,k] + b[i,j,h]
  A = softmax_j(attn);  out[n,i,:] = (gate * einsum(A, v)) @ Wo^T + bo
  result[i,n,:] = out[n,i,:]

Sharding: n split 48 rows/core. Each core computes partial logits for its
n-slice; a ReduceScatter (over i) sums them, each core adds its LOCAL B
i-slab (B[i] lives on the core that owns bias rows i), softmaxes its 48
i-rows, and an AllGather distributes the attention matrix A to all cores.

v2 changes vs v1 (549us -> target):
  - host pre-casts pair/bias to bf16 (halves input DMA), output in bf16
    (halves store DMA), upcast on host
  - CH=12 LN chunks (one q/k n-group per chunk), 3D batched bn_stats,
    transposes batched 3-per-psum with single [128,384] evictions
  - Q/K projected DIRECTLY into the packed (nn,k) logit layout with
    col-tiled M=32 matmuls (tile_position) -- no SBUF restack DMAs
  - AllReduce+AllGather(B) replaced by ReduceScatter+local-B+AllGather(A):
    softmax work /8, B never leaves its core, one less collective payload
  - A^T built by 12 xbar DMA-transposes straight from the AllGather DRAM
    output (no PE transposes / evictions in the tail)
  - einsum A@V col-tiled per head -> full-width [128,384] evictions into
    gated[hd, (n,i)] (v1 scattered 32-partition copies were the tail's
    bottleneck)
  - V-proj and out-proj batched 4 matmuls/psum-bank, single 512-wide evicts
  - out-proj bias bo added as one rank-1 K=1 matmul per psum bank
"""
import sys

if "/opt/trn_rl_repo" not in sys.path:
    sys.path.insert(0, "/opt/trn_rl_repo")

import numpy as np
import ml_dtypes
from contextlib import ExitStack

import concourse.bass as bass
import concourse.bacc as bacc
import concourse.mybir as mybir
import concourse.tile as tile
from concourse.bass_utils import run_bass_kernel_spmd
from concourse.masks import make_identity

F32 = mybir.dt.float32
BF16 = mybir.dt.bfloat16
AF = mybir.ActivationFunctionType
ALU = mybir.AluOpType
AX = mybir.AxisListType

O, L, DP, H, DH = 1, 384, 128, 4, 32
HD = H * DH
NCORES = 8
NS = L // NCORES            # 48 n's per core
R = NS * L                  # 18432 rows per core
NBLK = R // 128             # 144 row-blocks
NG = NS // 4                # 12 logit contraction groups (4 n's each)
IB = L // 128               # 3 blocks of 128 along i/j
SCALING = 1.0 / np.sqrt(DH)
EPS = 1e-5

CH = 12                     # LN chunk: 12 blocks = 1536 rows = 4 n (1 group)
NCHUNK = NBLK // CH         # 12 chunks per tensor


def build_tile_kernel(ctx: ExitStack, tc: tile.TileContext, outs, ins):
    nc = tc.nc
    pairc = ins["pairc"].rearrange("(b p) d -> p b d", p=128)   # [128,144,128]
    biasc = ins["biasc"].rearrange("(b p) d -> p b d", p=128)
    outc = outs["outc"].rearrange("(b p) d -> p b d", p=128)

    const = ctx.enter_context(tc.tile_pool(name="const", bufs=1))
    big = ctx.enter_context(tc.tile_pool(name="big", bufs=1))
    stream = ctx.enter_context(tc.tile_pool(name="stream", bufs=2))
    lnload = ctx.enter_context(tc.tile_pool(name="lnload", bufs=4))
    btcp = ctx.enter_context(tc.tile_pool(name="btcp", bufs=3))
    outstg = ctx.enter_context(tc.tile_pool(name="outstg", bufs=3))
    smx = ctx.enter_context(tc.tile_pool(name="smx", bufs=1))
    ps_tr = ctx.enter_context(tc.tile_pool(name="pstr", bufs=3, space="PSUM"))
    ps_acc = ctx.enter_context(tc.tile_pool(name="psacc", bufs=3, space="PSUM"))
    ps_pj = ctx.enter_context(tc.tile_pool(name="pspj", bufs=2, space="PSUM"))
    dram = ctx.enter_context(tc.tile_pool(name="dram", bufs=1, space="DRAM"))

    # ---------------- constants / weights ----------------------------------
    ident = const.tile([128, 128], BF16)
    make_identity(nc, ident)
    epst = const.tile([128, 1], F32)
    nc.vector.memset(epst[:], EPS)
    onesrow = const.tile([1, 128], BF16)
    nc.vector.memset(onesrow[:], 1.0)

    vecs = const.tile([128, 4], F32)          # cols: gp, gb, bg, bo
    nc.sync.dma_start(vecs[:], ins["vecs"][:])
    borow = const.tile([1, 128], F32)
    nc.sync.dma_start(borow[:], ins["borow"][:])
    borow4 = const.tile([1, 4, 128], BF16)
    for q4 in range(4):
        nc.vector.tensor_copy(borow4[:, q4, :], borow[:])

    wf = {}
    for nm in ("wqt", "wkt", "wvt", "wgt", "wot"):
        t = const.tile([128, 128], F32, name=f"{nm}_f")
        nc.sync.dma_start(t[:], ins[nm][:])
        wf[nm] = t
    wbt_f = const.tile([128, 4], F32)
    nc.sync.dma_start(wbt_f[:], ins["wbt"][:])

    # fold LN gain + scale constants into bf16 lhsT weights (per-partition d)
    wb = {}
    for nm, extra in (("wqt", SCALING), ("wkt", 1.0 / L), ("wvt", 1.0),
                      ("wgt", 1.0)):
        gs = const.tile([128, 1], F32, name=f"{nm}_gs")
        nc.vector.tensor_scalar_mul(gs[:], vecs[:, 0:1], extra)
        t = const.tile([128, 128], BF16, name=f"{nm}_b")
        nc.vector.tensor_scalar_mul(t[:], wf[nm][:], gs[:, 0:1])
        wb[nm] = t
    wot_b = const.tile([128, 128], BF16)
    nc.vector.tensor_copy(wot_b[:], wf["wot"][:])
    wbt_b = const.tile([128, 4], BF16)
    nc.vector.tensor_scalar_mul(wbt_b[:], wbt_f[:], vecs[:, 1:2])

    # ---------------- persistent SBUF / DRAM tensors -----------------------
    xt = big.tile([128, NS, L], BF16, tag="xt")        # X^T [d,(n,i)]
    # packed logit layout: partition p = 32*nn + k  (nn-outer, k-inner)
    qs = big.tile([128, H, NG, L], BF16, tag="qs")     # [(nn,k), h, g, i]
    ks = big.tile([128, H, NG, L], BF16, tag="ks")
    zsb = big.tile([128, IB, H, L], BF16, tag="z")     # logits [i%128, ib, h, j]
    at = big.tile([128, H, IB, L], BF16, tag="z",
                  name="at")                           # A^T; reuses zsb slot

    zin = dram.tile([L, H * L], BF16)                  # Z rows = i
    zred = dram.tile([NS, H * L], BF16)                # RS out: my i-rows
    bgin = dram.tile([NS, H, L], F32)                  # local B [il, h, j]
    agin = dram.tile([NS, H * L], BF16)                # my softmaxed A rows
    agout = dram.tile([L, H, L], BF16, addr_space="Shared")    # full A [i, h, j]

    # ---------------- LayerNorm streaming ----------------------------------
    def ln_chunk(src_dram, ch, dst_cb):
        """Load CH=12 row-blocks, LN them, emit 4 transposed [128,384] psum
        tiles (one per n/il row) to dst_cb(row_idx, pt)."""
        rm = lnload.tile([128, CH, 128], BF16, tag="lnrm")
        nc.gpsimd.dma_start(rm[:], src_dram[:, ch * CH:(ch + 1) * CH, :])
        st = stream.tile([128, CH, 6], F32, tag="lnst")
        for b in range(CH):
            nc.vector.bn_stats(st[:, b, :], rm[:, b, :])
        v0 = stream.tile([128, CH], F32, tag="lnv0")
        dm = stream.tile([128, CH], F32, tag="lndm")
        r = stream.tile([128, CH], F32, tag="lnr")
        nmr = stream.tile([128, CH], F32, tag="lnnmr")
        # var = (cv_e+cv_o)/128 + ((m_e-m_o)/2)^2 ; mean = (m_e+m_o)/2
        nc.vector.tensor_tensor(v0[:], st[:, :, 2], st[:, :, 5], ALU.add)
        nc.vector.tensor_scalar_mul(v0[:], v0[:], 1.0 / 128)
        nc.vector.tensor_tensor(dm[:], st[:, :, 1], st[:, :, 4], ALU.subtract)
        nc.vector.tensor_tensor(dm[:], dm[:], dm[:], ALU.mult)
        nc.vector.tensor_scalar_mul(dm[:], dm[:], 0.25)
        nc.vector.tensor_tensor(v0[:], v0[:], dm[:], ALU.add)
        nc.scalar.activation(r[:], v0[:], AF.Sqrt, bias=epst[:, 0:1])
        nc.vector.reciprocal(r[:], r[:])
        nc.vector.tensor_tensor(nmr[:], st[:, :, 1], st[:, :, 4], ALU.add)
        nc.vector.tensor_tensor(nmr[:], nmr[:], r[:], ALU.mult)
        nc.vector.tensor_scalar_mul(nmr[:], nmr[:], -0.5)
        norm = stream.tile([128, CH, 128], BF16, tag="lnnorm")
        for b in range(CH):
            if b % 2 == 0:
                nc.scalar.activation(norm[:, b, :], rm[:, b, :], AF.Identity,
                                     bias=nmr[:, b:b + 1], scale=r[:, b:b + 1])
            else:
                nc.vector.tensor_scalar(norm[:, b, :], rm[:, b, :],
                                        r[:, b:b + 1], nmr[:, b:b + 1],
                                        ALU.mult, ALU.add)
        for q in range(CH // 3):
            pt = ps_tr.tile([128, 3 * 128], BF16, tag="tr")
            for b3 in range(3):
                nc.tensor.transpose(pt[:, b3 * 128:(b3 + 1) * 128],
                                    norm[:, 3 * q + b3, :], ident[:])
            dst_cb(ch * 4 + q, pt)

    def pair_dst(nrow, pt):
        if nrow % 2 == 0:
            nc.vector.tensor_copy(xt[:, nrow, :], pt[:])
        else:
            nc.scalar.copy(xt[:, nrow, :], pt[:])

    # ---------------- phase A: pair LN + packed Q/K projections ------------
    # qs/ks partition layout (nn-outer, k-inner) is produced directly by
    # col-tiled matmuls: for head h, n-sub nn, the M=32 weight slice
    # w[:, 32h:32h+32] lands on psum partitions [32nn, 32nn+32).
    for g in range(NG):
        ln_chunk(pairc, g, pair_dst)
        for idx, (wname, dst) in enumerate((("wqt", qs), ("wkt", ks))):
            pq = ps_acc.tile([128, L], F32, tag="acc")
            for h in range(H):
                for nn in range(4):
                    nc.tensor.matmul(pq[32 * nn:32 * nn + 32, :],
                                     wb[wname][:, 32 * h:32 * h + 32],
                                     xt[:, 4 * g + nn, :], start=True,
                                     stop=True, tile_position=(0, 32 * nn))
                if (h + idx) % 2 == 0:
                    nc.vector.tensor_copy(dst[:, h, g, :], pq[:])
                else:
                    nc.scalar.copy(dst[:, h, g, :], pq[:])

    # ---------------- bias LN chunks interleaved with logits ---------------
    # bias chunk u covers i-rows 4u..4u+3; logits unit u is (h,ib) pair u.
    # Emitting them alternately lets ACT/DVE (bias LN) run under the PE
    # (logits) with the Tile scheduler pipelining both.
    def bias_dst_factory(btc):
        def bias_dst(ilrow, pt, btc=btc):
            il = ilrow % 4
            if il % 2 == 0:
                nc.vector.tensor_copy(btc[:, il, :], pt[:])
            else:
                nc.scalar.copy(btc[:, il, :], pt[:])
        return bias_dst

    for u in range(NCHUNK):
        btc = btcp.tile([128, 4, L], BF16, tag="btc")
        ln_chunk(biasc, u, bias_dst_factory(btc))
        bst = btcp.tile([4, 4, L], F32, tag="bstg")
        for il in range(4):
            bp = ps_pj.tile([4, L], F32, tag="pj4")
            nc.tensor.matmul(bp[:], wbt_b[:], btc[:, il, :], start=True,
                             stop=True)
            nc.scalar.copy(bst[:, il, :], bp[:])
        nc.sync.dma_start(bgin[4 * u:4 * u + 4, :, :].transpose([1, 0, 2]),
                          bst[:])
        # one logits (h, ib) unit
        h, ib = u // IB, u % IB
        lp = ps_acc.tile([128, L], F32, tag="acc")
        for g in range(NG):
            nc.tensor.matmul(lp[:], qs[:, h, g, ib * 128:(ib + 1) * 128],
                             ks[:, h, g, :], start=(g == 0),
                             stop=(g == NG - 1))
        if u % 2 == 0:
            nc.vector.tensor_copy(zsb[:, ib, h, :], lp[:])
        else:
            nc.scalar.copy(zsb[:, ib, h, :], lp[:])

    # Z -> DRAM [i, (h j)] and ReduceScatter over i (rank c owns i-rows
    # [48c, 48c+48) -- the same i's whose B slab this core computed)
    zin_v = zin[:].rearrange("(ib p) f -> p ib f", p=128)
    nc.sync.dma_start(zin_v, zsb[:].rearrange("p a b c -> p a (b c)"))
    nc.gpsimd.collective_compute(
        "ReduceScatter", ALU.add, replica_groups=[list(range(NCORES))],
        ins=[zin[:].opt()], outs=[zred[:].opt()])

    # ---------------- G + V projections (overlap the ReduceScatter) --------
    gsb = big.tile([128, NS, L], BF16, tag="ks")       # reuses ks slot
    for n in range(NS):
        gp_ = ps_acc.tile([128, L], F32, tag="acc")
        nc.tensor.matmul(gp_[:], wb["wgt"][:], xt[:, n, :], start=True,
                         stop=True)
        nc.scalar.activation(gsb[:, n, :], gp_[:], AF.Sigmoid,
                             bias=vecs[:, 2:3])

    # vt layout [j%128, jb, n, hd]: einsum stationary slice
    # vt[:, jb, n, 32h:32h+32] is one contiguous free run
    vt = big.tile([128, IB, NS, HD], BF16, tag="qs")   # reuses qs slot
    for jb in range(IB):
        for n0 in range(0, NS, 4):
            vp = ps_acc.tile([128, 4, 128], F32, tag="acc")
            for q in range(4):
                nc.tensor.matmul(vp[:, q, :],
                                 xt[:, n0 + q, jb * 128:(jb + 1) * 128],
                                 wb["wvt"][:], start=True, stop=True)
            if (jb + n0 // 4) % 2 == 0:
                nc.vector.tensor_copy(vt[:, jb, n0:n0 + 4, :], vp[:])
            else:
                nc.scalar.copy(vt[:, jb, n0:n0 + 4, :], vp[:])

    # ---------------- local B add + softmax on my 48 i-rows ----------------
    zl = smx.tile([48, H * L], BF16, tag="zl")
    nc.sync.dma_start(zl[:], zred[:])
    bl = smx.tile([48, H, L], F32, tag="bl")
    nc.sync.dma_start(bl[:], bgin[:])
    sm = smx.tile([48, H, L], F32, tag="sm")
    nc.vector.tensor_tensor(sm[:].rearrange("p a b -> p (a b)"), zl[:],
                            bl[:].rearrange("p a b -> p (a b)"), ALU.add)
    # logits are O(1): softmax without max subtraction is exact in f32
    ssum = smx.tile([48, H], F32, tag="ssum")
    for h in range(H):
        nc.scalar.activation(sm[:, h, :], sm[:, h, :], AF.Exp,
                             accum_out=ssum[:, h:h + 1])
    rr = smx.tile([48, H], F32, tag="rr")
    nc.vector.reciprocal(rr[:], ssum[:])
    aout = smx.tile([48, H, L], BF16, tag="aout")
    for h in range(H):
        nc.vector.tensor_scalar_mul(aout[:, h, :], sm[:, h, :],
                                    rr[:, h:h + 1])
    nc.sync.dma_start(agin[:], aout[:].rearrange("p a b -> p (a b)"))
    nc.gpsimd.collective_compute(
        "AllGather", ALU.bypass, replica_groups=[list(range(NCORES))],
        ins=[agin[:].opt()], outs=[agout[:].opt()])

    # A^T tiles straight from DRAM via xbar transpose: [384 i, 128 j] ->
    # [128 j, 384 i] per (h, jb)
    for h in range(H):
        for jb in range(IB):
            nc.sync.dma_start_transpose(
                at[:, h, jb, :], agout[:, h, jb * 128:(jb + 1) * 128])

    # ---------------- einsum A@V (col-tiled per head) + gate + out-proj ----
    gated = big.tile([128, NS, L], BF16, tag="xt")     # reuses xt slot
    gated_flat = gated[:].rearrange("p n l -> p (n l)")
    for n8 in range(NS // 8):
        for nn in range(8):
            n = 8 * n8 + nn
            ep = ps_acc.tile([128, L], F32, tag="acc")
            for h in range(H):
                for jb in range(IB):
                    nc.tensor.matmul(ep[32 * h:32 * h + 32, :],
                                     vt[:, jb, n, 32 * h:32 * h + 32],
                                     at[:, h, jb, :], start=(jb == 0),
                                     stop=(jb == IB - 1),
                                     tile_position=(0, 32 * h))
            if n % 2 == 0:
                nc.vector.tensor_copy(gated[:, n, :], ep[:])
            else:
                nc.scalar.copy(gated[:, n, :], ep[:])
        # full-width gate for these 8 n's
        nc.vector.tensor_tensor(
            gated[:, 8 * n8:8 * n8 + 8, :], gated[:, 8 * n8:8 * n8 + 8, :],
            gsb[:, 8 * n8:8 * n8 + 8, :], ALU.mult)
        # output projection for the 24 row-blocks of these 8 n's
        for fb in range(6):
            fp = ps_acc.tile([128, 4, 128], F32, tag="acc")
            # bo via one rank-1 K=1 matmul across all 4 blocks
            nc.tensor.matmul(fp[:].rearrange("p a b -> p (a b)"), onesrow[:],
                             borow4[:].rearrange("p a b -> p (a b)"),
                             start=True, stop=False)
            for q in range(4):
                rb = n8 * 24 + fb * 4 + q
                nc.tensor.matmul(fp[:, q, :],
                                 gated_flat[:, rb * 128:(rb + 1) * 128],
                                 wot_b[:], start=False, stop=(q == 3))
            fst = outstg.tile([128, 4, 128], BF16, tag="fstg")
            if fb % 2 == 0:
                nc.vector.tensor_copy(fst[:], fp[:])
            else:
                nc.scalar.copy(fst[:], fp[:])
            fbg = n8 * 6 + fb
            nc.sync.dma_start(outc[:, fbg * 4:(fbg + 1) * 4, :], fst[:])


# ---------------------------------------------------------------------------
_NC_CACHE = {}


def _build_program():
    if "nc" in _NC_CACHE:
        return _NC_CACHE["nc"]
    nc = bacc.Bacc("TRN2", target_bir_lowering=False, debug=False,
                   enable_asserts=False, num_devices=NCORES)
    ins = {
        "pairc": nc.dram_tensor("pairc", [R, DP], BF16, kind="ExternalInput").ap(),
        "biasc": nc.dram_tensor("biasc", [R, DP], BF16, kind="ExternalInput").ap(),
        "wqt": nc.dram_tensor("wqt", [DP, HD], F32, kind="ExternalInput").ap(),
        "wkt": nc.dram_tensor("wkt", [DP, HD], F32, kind="ExternalInput").ap(),
        "wvt": nc.dram_tensor("wvt", [DP, HD], F32, kind="ExternalInput").ap(),
        "wgt": nc.dram_tensor("wgt", [DP, HD], F32, kind="ExternalInput").ap(),
        "wot": nc.dram_tensor("wot", [HD, DP], F32, kind="ExternalInput").ap(),
        "wbt": nc.dram_tensor("wbt", [DP, H], F32, kind="ExternalInput").ap(),
        "vecs": nc.dram_tensor("vecs", [DP, 4], F32, kind="ExternalInput").ap(),
        "borow": nc.dram_tensor("borow", [1, DP], F32, kind="ExternalInput").ap(),
    }
    outs = {
        "outc": nc.dram_tensor("outc", [R, DP], BF16, kind="ExternalOutput").ap(),
    }
    with tile.TileContext(nc) as tc:
        with ExitStack() as ctx:
            build_tile_kernel(ctx, tc, outs, ins)
    nc.compile()
    _NC_CACHE["nc"] = nc
    return nc


def shard_inputs(pair, bias, ln_pair_g, ln_pair_b, ln_bias_g, ln_bias_b,
                 Wq, Wk, Wv, Wb, Wg, bg, Wo, bo):
    """Host-side slicing/permutation -> per-core input maps."""
    assert pair.shape == (O, L, L, DP) and bias.shape == (O, L, L, DP)
    assert np.abs(ln_pair_b).max() == 0 and np.abs(ln_bias_b).max() == 0, \
        "kernel folds LN beta=0; nonzero beta not implemented"
    f32 = np.float32
    bf16 = ml_dtypes.bfloat16
    shared = {
        "wqt": np.ascontiguousarray(Wq.T, f32),
        "wkt": np.ascontiguousarray(Wk.T, f32),
        "wvt": np.ascontiguousarray(Wv.T, f32),
        "wgt": np.ascontiguousarray(Wg.T, f32),
        "wot": np.ascontiguousarray(Wo.T, f32),
        "wbt": np.ascontiguousarray(Wb.T, f32),
        "vecs": np.ascontiguousarray(
            np.stack([ln_pair_g, ln_bias_g, bg, bo], axis=1), f32),
        "borow": np.ascontiguousarray(bo[None, :], f32),
    }
    pair_b = pair[0].astype(bf16)
    bias_b = bias[0].astype(bf16)
    in_maps = []
    for c in range(NCORES):
        S = slice(c * NS, (c + 1) * NS)
        m = dict(shared)
        m["pairc"] = np.ascontiguousarray(
            pair_b[:, S, :].transpose(1, 0, 2).reshape(R, DP))
        m["biasc"] = np.ascontiguousarray(
            bias_b[:, S, :].transpose(1, 0, 2).reshape(R, DP))
        in_maps.append(m)
    return in_maps


def gather_outputs(results):
    res = np.zeros((O, L, L, DP), np.float32)
    for c in range(NCORES):
        F = results[c]["outc"].astype(np.float32).reshape(NS, L, DP)
        res[0, :, c * NS:(c + 1) * NS, :] = F.transpose(1, 0, 2)
    return res


def kernel(**inputs):
    inputs = {k: np.asarray(v) for k, v in inputs.items()}
    nc = _build_program()
    in_maps = shard_inputs(**inputs)
    r = run_bass_kernel_spmd(nc, in_maps, core_ids=list(range(NCORES)))
    return gather_outputs(r.results)


def _ensure_ntff_hook():
    """The agent image's antenv lacks axon_hooks; recreate the registry and
    wire the ctypes NTFF hook from trn_agent_boot (profiling-only path)."""
    try:
        from antenv.axon_hooks import get_axon_ntff_profile_hook  # noqa: F401
        return
    except ImportError:
        pass
    import types
    import antenv
    mod = types.ModuleType("antenv.axon_hooks")
    mod._hook = None
    mod.set_axon_ntff_profile_hook = lambda h: setattr(mod, "_hook", h)
    mod.get_axon_ntff_profile_hook = lambda: mod._hook
    sys.modules["antenv.axon_hooks"] = mod
    antenv.axon_hooks = mod
    try:
        from trn_agent_boot.trn_boot import _ntff_profile_via_ctypes
        hook = _ntff_profile_via_ctypes("/opt/axon/libaxon_pjrt.so")
        if hook is not None:
            mod._hook = hook
    except Exception as e:  # profiling degrades, run still works
        print(f"NTFF hook setup failed: {e}", file=sys.stderr)


def kernel_profiled(**inputs):
    """Like kernel() but also returns exec-time info from neuron-profile."""
    inputs = {k: np.asarray(v) for k, v in inputs.items()}
    _ensure_ntff_hook()
    import concourse.bass_utils as bu
    bu.upload_artifacts = lambda tmpdir: f"local:{tmpdir}"  # no bucket here
    nc = _build_program()
    in_maps = shard_inputs(**inputs)
    r = run_bass_kernel_spmd(nc, in_maps, core_ids=list(range(NCORES)),
                             trace=True, trace_cores=list(range(NCORES)))
    return gather_outputs(r.results), r


# revision 11
# speedup vs baseline: 1.3640x; 1.0355x over previous
"""Biased axial attention (RoseTTAFold-style) on 8 TRN2 NeuronCores — v2.

nn_BiasedAxialAttention: O=1, L=384, d_pair=d_bias=128, H=4, DH=32.

  p    = LN(pair^T);  bsrc = LN(bias^T)            (LN over d per position)
  q,k,v,gate projections of p; b = bsrc @ Wb^T
  attn[i,j,h] = sum_{n,k} q[n,i,h,k] k[n,j,h,k] + b[i,j,h]
  A = softmax_j(attn);  out[n,i,:] = (gate * einsum(A, v)) @ Wo^T + bo
  result[i,n,:] = out[n,i,:]

Sharding: n split 48 rows/core. Each core computes partial logits for its
n-slice; a ReduceScatter (over i) sums them, each core adds its LOCAL B
i-slab (B[i] lives on the core that owns bias rows i), softmaxes its 48
i-rows, and an AllGather distributes the attention matrix A to all cores.

v2 changes vs v1 (549us -> target):
  - host pre-casts pair/bias to bf16 (halves input DMA), output in bf16
    (halves store DMA), upcast on host
  - CH=12 LN chunks (one q/k n-group per chunk), 3D batched bn_stats,
    transposes batched 3-per-psum with single [128,384] evictions
  - Q/K projected DIRECTLY into the packed (nn,k) logit layout with
    col-tiled M=32 matmuls (tile_position) -- no SBUF restack DMAs
  - AllReduce+AllGather(B) replaced by ReduceScatter+local-B+AllGather(A):
    softmax work /8, B never leaves its core, one less collective payload
  - A^T built by 12 xbar DMA-transposes straight from the AllGather DRAM
    output (no PE transposes / evictions in the tail)
  - einsum A@V col-tiled per head -> full-width [128,384] evictions into
    gated[hd, (n,i)] (v1 scattered 32-partition copies were the tail's
    bottleneck)
  - V-proj and out-proj batched 4 matmuls/psum-bank, single 512-wide evicts
  - out-proj bias bo added as one rank-1 K=1 matmul per psum bank
"""
import sys

if "/opt/trn_rl_repo" not in sys.path:
    sys.path.insert(0, "/opt/trn_rl_repo")

import numpy as np
import ml_dtypes
from contextlib import ExitStack

import concourse.bass as bass
import concourse.bacc as bacc
import concourse.mybir as mybir
import concourse.tile as tile
from concourse.bass_utils import run_bass_kernel_spmd
from concourse.masks import make_identity

F32 = mybir.dt.float32
BF16 = mybir.dt.bfloat16
AF = mybir.ActivationFunctionType
ALU = mybir.AluOpType
AX = mybir.AxisListType

O, L, DP, H, DH = 1, 384, 128, 4, 32
HD = H * DH
NCORES = 8
NS = L // NCORES            # 48 n's per core
R = NS * L                  # 18432 rows per core
NBLK = R // 128             # 144 row-blocks
NG = NS // 4                # 12 logit contraction groups (4 n's each)
IB = L // 128               # 3 blocks of 128 along i/j
SCALING = 1.0 / np.sqrt(DH)
EPS = 1e-5

CH = 12                     # LN chunk: 12 blocks = 1536 rows = 4 n (1 group)
NCHUNK = NBLK // CH         # 12 chunks per tensor


def build_tile_kernel(ctx: ExitStack, tc: tile.TileContext, outs, ins):
    nc = tc.nc
    pairc = ins["pairc"].rearrange("(b p) d -> p b d", p=128)   # [128,144,128]
    biasc = ins["biasc"].rearrange("(b p) d -> p b d", p=128)
    outc = outs["outc"]                                # [DP, R] transposed

    const = ctx.enter_context(tc.tile_pool(name="const", bufs=1))
    big = ctx.enter_context(tc.tile_pool(name="big", bufs=1))
    stream = ctx.enter_context(tc.tile_pool(name="stream", bufs=2))
    lnload = ctx.enter_context(tc.tile_pool(name="lnload", bufs=4))
    btcp = ctx.enter_context(tc.tile_pool(name="btcp", bufs=3))
    outstg = ctx.enter_context(tc.tile_pool(name="outstg", bufs=3))
    smx = ctx.enter_context(tc.tile_pool(name="smx", bufs=1))
    ps_tr = ctx.enter_context(tc.tile_pool(name="pstr", bufs=3, space="PSUM"))
    ps_acc = ctx.enter_context(tc.tile_pool(name="psacc", bufs=3, space="PSUM"))
    ps_pj = ctx.enter_context(tc.tile_pool(name="pspj", bufs=2, space="PSUM"))
    dram = ctx.enter_context(tc.tile_pool(name="dram", bufs=1, space="DRAM"))

    # ---------------- constants / weights ----------------------------------
    ident = const.tile([128, 128], BF16)
    make_identity(nc, ident)
    epst = const.tile([128, 1], F32)
    nc.vector.memset(epst[:], EPS)
    vecs = const.tile([128, 4], F32)          # cols: gp, gb, bg, bo
    nc.sync.dma_start(vecs[:], ins["vecs"][:])

    wf = {}
    for nm in ("wqt", "wkt", "wvt", "wgt", "wot"):
        t = const.tile([128, 128], F32, name=f"{nm}_f")
        nc.sync.dma_start(t[:], ins[nm][:])
        wf[nm] = t
    wbt_f = const.tile([128, 4], F32)
    nc.sync.dma_start(wbt_f[:], ins["wbt"][:])

    # fold LN gain + scale constants into bf16 lhsT weights (per-partition d)
    wb = {}
    for nm, extra in (("wqt", SCALING), ("wkt", 1.0 / L), ("wvt", 1.0),
                      ("wgt", 1.0)):
        gs = const.tile([128, 1], F32, name=f"{nm}_gs")
        nc.vector.tensor_scalar_mul(gs[:], vecs[:, 0:1], extra)
        t = const.tile([128, 128], BF16, name=f"{nm}_b")
        nc.vector.tensor_scalar_mul(t[:], wf[nm][:], gs[:, 0:1])
        wb[nm] = t
    wot_b = const.tile([128, 128], BF16)
    nc.vector.tensor_copy(wot_b[:], wf["wot"][:])
    wbt_b = const.tile([128, 4], BF16)
    nc.vector.tensor_scalar_mul(wbt_b[:], wbt_f[:], vecs[:, 1:2])

    # ---------------- persistent SBUF / DRAM tensors -----------------------
    xt = big.tile([128, NS, L], BF16, tag="xt")        # X^T [d,(n,i)]
    # packed logit layout: partition p = 32*nn + k  (nn-outer, k-inner)
    qs = big.tile([128, H, NG, L], BF16, tag="qs")     # [(nn,k), h, g, i]
    ks = big.tile([128, H, NG, L], BF16, tag="ks")
    zsb = big.tile([128, IB, H, L], BF16, tag="z")     # logits [i%128, ib, h, j]
    at = big.tile([128, H, IB, L], BF16, tag="z",
                  name="at")                           # A^T; reuses zsb slot

    zin = dram.tile([L, H * L], BF16)                  # Z rows = i
    zred = dram.tile([NS, H * L], BF16)                # RS out: my i-rows
    bgin = dram.tile([NS, H, L], F32)                  # local B [il, h, j]
    agin = dram.tile([NS, H * L], BF16)                # my softmaxed A rows
    agout = dram.tile([L, H, L], BF16, addr_space="Shared")    # full A [i, h, j]

    # ---------------- LayerNorm streaming ----------------------------------
    def ln_chunk(src_dram, ch, dst_cb):
        """Load CH=12 row-blocks, LN them, emit 4 transposed [128,384] psum
        tiles (one per n/il row) to dst_cb(row_idx, pt)."""
        rm = lnload.tile([128, CH, 128], BF16, tag="lnrm")
        nc.gpsimd.dma_start(rm[:], src_dram[:, ch * CH:(ch + 1) * CH, :])
        st = stream.tile([128, CH, 6], F32, tag="lnst")
        for b in range(CH):
            nc.vector.bn_stats(st[:, b, :], rm[:, b, :])
        v0 = stream.tile([128, CH], F32, tag="lnv0")
        dm = stream.tile([128, CH], F32, tag="lndm")
        r = stream.tile([128, CH], F32, tag="lnr")
        nmr = stream.tile([128, CH], F32, tag="lnnmr")
        # var = (cv_e+cv_o)/128 + ((m_e-m_o)/2)^2 ; mean = (m_e+m_o)/2
        nc.vector.tensor_tensor(v0[:], st[:, :, 2], st[:, :, 5], ALU.add)
        nc.vector.tensor_scalar_mul(v0[:], v0[:], 1.0 / 128)
        nc.vector.tensor_tensor(dm[:], st[:, :, 1], st[:, :, 4], ALU.subtract)
        nc.vector.tensor_tensor(dm[:], dm[:], dm[:], ALU.mult)
        nc.vector.tensor_scalar_mul(dm[:], dm[:], 0.25)
        nc.vector.tensor_tensor(v0[:], v0[:], dm[:], ALU.add)
        nc.scalar.activation(r[:], v0[:], AF.Sqrt, bias=epst[:, 0:1])
        nc.vector.reciprocal(r[:], r[:])
        nc.vector.tensor_tensor(nmr[:], st[:, :, 1], st[:, :, 4], ALU.add)
        nc.vector.tensor_tensor(nmr[:], nmr[:], r[:], ALU.mult)
        nc.vector.tensor_scalar_mul(nmr[:], nmr[:], -0.5)
        norm = stream.tile([128, CH, 128], BF16, tag="lnnorm")
        for b in range(CH):
            if b % 3 == 0:
                nc.scalar.activation(norm[:, b, :], rm[:, b, :], AF.Identity,
                                     bias=nmr[:, b:b + 1], scale=r[:, b:b + 1])
            else:
                nc.vector.tensor_scalar(norm[:, b, :], rm[:, b, :],
                                        r[:, b:b + 1], nmr[:, b:b + 1],
                                        ALU.mult, ALU.add)
        for q in range(CH // 3):
            pt = ps_tr.tile([128, 3 * 128], BF16, tag="tr")
            for b3 in range(3):
                nc.tensor.transpose(pt[:, b3 * 128:(b3 + 1) * 128],
                                    norm[:, 3 * q + b3, :], ident[:])
            dst_cb(ch * 4 + q, pt)

    def pair_dst(nrow, pt):
        if nrow % 2 == 0:
            nc.vector.tensor_copy(xt[:, nrow, :], pt[:])
        else:
            nc.scalar.copy(xt[:, nrow, :], pt[:])

    # ---------------- phase A: pair LN + packed Q/K projections ------------
    # qs/ks partition layout (nn-outer, k-inner) is produced directly by
    # col-tiled matmuls: for head h, n-sub nn, the M=32 weight slice
    # w[:, 32h:32h+32] lands on psum partitions [32nn, 32nn+32).
    for g in range(NG):
        ln_chunk(pairc, g, pair_dst)
        for idx, (wname, dst) in enumerate((("wqt", qs), ("wkt", ks))):
            pq = ps_acc.tile([128, L], F32, tag="acc")
            for h in range(H):
                for nn in range(4):
                    nc.tensor.matmul(pq[32 * nn:32 * nn + 32, :],
                                     wb[wname][:, 32 * h:32 * h + 32],
                                     xt[:, 4 * g + nn, :], start=True,
                                     stop=True, tile_position=(0, 32 * nn))
                if (h + idx) % 2 == 0:
                    nc.vector.tensor_copy(dst[:, h, g, :], pq[:])
                else:
                    nc.scalar.copy(dst[:, h, g, :], pq[:])

    # ---------------- bias LN chunks interleaved with logits ---------------
    # bias chunk u covers i-rows 4u..4u+3; logits unit u is (h,ib) pair u.
    # Emitting them alternately lets ACT/DVE (bias LN) run under the PE
    # (logits) with the Tile scheduler pipelining both.
    def bias_dst_factory(btc):
        def bias_dst(ilrow, pt, btc=btc):
            il = ilrow % 4
            if il % 2 == 0:
                nc.vector.tensor_copy(btc[:, il, :], pt[:])
            else:
                nc.scalar.copy(btc[:, il, :], pt[:])
        return bias_dst

    for u in range(NCHUNK):
        btc = btcp.tile([128, 4, L], BF16, tag="btc")
        ln_chunk(biasc, u, bias_dst_factory(btc))
        bst = btcp.tile([4, 4, L], F32, tag="bstg")
        for il in range(4):
            bp = ps_pj.tile([4, L], F32, tag="pj4")
            nc.tensor.matmul(bp[:], wbt_b[:], btc[:, il, :], start=True,
                             stop=True)
            nc.scalar.copy(bst[:, il, :], bp[:])
        nc.sync.dma_start(bgin[4 * u:4 * u + 4, :, :].transpose([1, 0, 2]),
                          bst[:])
        # one logits (h, ib) unit
        h, ib = u // IB, u % IB
        lp = ps_acc.tile([128, L], F32, tag="acc")
        for g in range(NG):
            nc.tensor.matmul(lp[:], qs[:, h, g, ib * 128:(ib + 1) * 128],
                             ks[:, h, g, :], start=(g == 0),
                             stop=(g == NG - 1))
        if u % 2 == 0:
            nc.vector.tensor_copy(zsb[:, ib, h, :], lp[:])
        else:
            nc.scalar.copy(zsb[:, ib, h, :], lp[:])

    # Z -> DRAM [i, (h j)] and ReduceScatter over i (rank c owns i-rows
    # [48c, 48c+48) -- the same i's whose B slab this core computed)
    zin_v = zin[:].rearrange("(ib p) f -> p ib f", p=128)
    nc.sync.dma_start(zin_v, zsb[:].rearrange("p a b c -> p a (b c)"))
    nc.gpsimd.collective_compute(
        "ReduceScatter", ALU.add, replica_groups=[list(range(NCORES))],
        ins=[zin[:].opt()], outs=[zred[:].opt()])

    # ---------------- local B add + softmax on my 48 i-rows ----------------
    zl = smx.tile([48, H * L], BF16, tag="zl")
    nc.gpsimd.dma_start(zl[:], zred[:])
    bl = smx.tile([48, H, L], F32, tag="bl")
    nc.gpsimd.dma_start(bl[:], bgin[:])
    sm = smx.tile([48, H, L], F32, tag="sm")
    nc.vector.tensor_tensor(sm[:].rearrange("p a b -> p (a b)"), zl[:],
                            bl[:].rearrange("p a b -> p (a b)"), ALU.add)
    # logits are O(1): softmax without max subtraction is exact in f32
    ssum = smx.tile([48, H], F32, tag="ssum")
    for h in range(H):
        nc.scalar.activation(sm[:, h, :], sm[:, h, :], AF.Exp,
                             accum_out=ssum[:, h:h + 1])
    rr = smx.tile([48, H], F32, tag="rr")
    nc.vector.reciprocal(rr[:], ssum[:])
    aout = smx.tile([48, H, L], BF16, tag="aout")
    for h in range(H):
        nc.vector.tensor_scalar_mul(aout[:, h, :], sm[:, h, :],
                                    rr[:, h:h + 1])
    nc.gpsimd.dma_start(agin[:], aout[:].rearrange("p a b -> p (a b)"))
    nc.gpsimd.collective_compute(
        "AllGather", ALU.bypass, replica_groups=[list(range(NCORES))],
        ins=[agin[:].opt()], outs=[agout[:].opt()])

    # A^T tiles straight from DRAM via xbar transpose: [384 i, 128 j] ->
    # [128 j, 384 i] per (h, jb)
    for h in range(H):
        for jb in range(IB):
            nc.sync.dma_start_transpose(
                at[:, h, jb, :], agout[:, h, jb * 128:(jb + 1) * 128])

    # ---------------- G + V projections (overlap the ReduceScatter) --------
    gsb = big.tile([128, NS, L], BF16, tag="ks")       # reuses ks slot
    for n in range(NS):
        gp_ = ps_acc.tile([128, L], F32, tag="acc")
        nc.tensor.matmul(gp_[:], wb["wgt"][:], xt[:, n, :], start=True,
                         stop=True)
        nc.scalar.activation(gsb[:, n, :], gp_[:], AF.Sigmoid,
                             bias=vecs[:, 2:3])

    # vt layout [j%128, jb, n, hd]: einsum stationary slice
    # vt[:, jb, n, 32h:32h+32] is one contiguous free run
    vt = big.tile([128, IB, NS, HD], BF16, tag="qs")   # reuses qs slot
    for jb in range(IB):
        for n0 in range(0, NS, 4):
            vp = ps_acc.tile([128, 4, 128], F32, tag="acc")
            for q in range(4):
                nc.tensor.matmul(vp[:, q, :],
                                 xt[:, n0 + q, jb * 128:(jb + 1) * 128],
                                 wb["wvt"][:], start=True, stop=True)
            if (jb + n0 // 4) % 2 == 0:
                nc.vector.tensor_copy(vt[:, jb, n0:n0 + 4, :], vp[:])
            else:
                nc.scalar.copy(vt[:, jb, n0:n0 + 4, :], vp[:])

    # ---------------- einsum A@V (col-tiled per head) + gate + out-proj ----
    # jb-outer/h-inner: the 4 head-quadrant matmuls of one jb round are
    # adjacent in the PE queue, so they execute concurrently (pc-monotone
    # starts); each quadrant's jb-accumulation chain overlaps the others.
    gated = big.tile([128, NS, L], BF16, tag="xt")     # reuses xt slot
    gated_flat = gated[:].rearrange("p n l -> p (n l)")
    for n8 in range(NS // 8):
        for nn in range(8):
            n = 8 * n8 + nn
            ep = ps_acc.tile([128, L], F32, tag="acc")
            for jb in range(IB):
                for h in range(H):
                    nc.tensor.matmul(ep[32 * h:32 * h + 32, :],
                                     vt[:, jb, n, 32 * h:32 * h + 32],
                                     at[:, h, jb, :], start=(jb == 0),
                                     stop=(jb == IB - 1),
                                     tile_position=(0, 32 * h),
                                     skip_group_check=True)
            if n % 2 == 0:
                nc.vector.tensor_copy(gated[:, n, :], ep[:])
            else:
                nc.scalar.copy(gated[:, n, :], ep[:])
        # full-width gate for these 8 n's
        nc.vector.tensor_tensor(
            gated[:, 8 * n8:8 * n8 + 8, :], gated[:, 8 * n8:8 * n8 + 8, :],
            gsb[:, 8 * n8:8 * n8 + 8, :], ALU.mult)
        # flipped output projection: stationary Wo^T, rhs = 512-wide gated
        # chunks, result [dp, rows] with bo as per-partition bias in the
        # eviction; stored transposed, host un-transposes
        for fb in range(6):
            ck = n8 * 6 + fb
            fp = ps_acc.tile([128, 512], F32, tag="acc")
            nc.tensor.matmul(fp[:], wot_b[:],
                             gated_flat[:, ck * 512:(ck + 1) * 512],
                             start=True, stop=True)
            fst = outstg.tile([128, 512], BF16, tag="fstg")
            if fb % 2 == 0:
                nc.vector.tensor_scalar_add(fst[:], fp[:], vecs[:, 3:4])
            else:
                nc.scalar.activation(fst[:], fp[:], AF.Identity,
                                     bias=vecs[:, 3:4])
            nc.sync.dma_start(outc[:, ck * 512:(ck + 1) * 512], fst[:])


# ---------------------------------------------------------------------------
_NC_CACHE = {}


def _build_program():
    if "nc" in _NC_CACHE:
        return _NC_CACHE["nc"]
    nc = bacc.Bacc("TRN2", target_bir_lowering=False, debug=False,
                   enable_asserts=False, num_devices=NCORES)
    ins = {
        "pairc": nc.dram_tensor("pairc", [R, DP], BF16, kind="ExternalInput").ap(),
        "biasc": nc.dram_tensor("biasc", [R, DP], BF16, kind="ExternalInput").ap(),
        "wqt": nc.dram_tensor("wqt", [DP, HD], F32, kind="ExternalInput").ap(),
        "wkt": nc.dram_tensor("wkt", [DP, HD], F32, kind="ExternalInput").ap(),
        "wvt": nc.dram_tensor("wvt", [DP, HD], F32, kind="ExternalInput").ap(),
        "wgt": nc.dram_tensor("wgt", [DP, HD], F32, kind="ExternalInput").ap(),
        "wot": nc.dram_tensor("wot", [HD, DP], F32, kind="ExternalInput").ap(),
        "wbt": nc.dram_tensor("wbt", [DP, H], F32, kind="ExternalInput").ap(),
        "vecs": nc.dram_tensor("vecs", [DP, 4], F32, kind="ExternalInput").ap(),
        "borow": nc.dram_tensor("borow", [1, DP], F32, kind="ExternalInput").ap(),
    }
    outs = {
        "outc": nc.dram_tensor("outc", [DP, R], BF16, kind="ExternalOutput").ap(),
    }
    with tile.TileContext(nc) as tc:
        with ExitStack() as ctx:
            build_tile_kernel(ctx, tc, outs, ins)
    nc.compile()
    _NC_CACHE["nc"] = nc
    return nc


def shard_inputs(pair, bias, ln_pair_g, ln_pair_b, ln_bias_g, ln_bias_b,
                 Wq, Wk, Wv, Wb, Wg, bg, Wo, bo):
    """Host-side slicing/permutation -> per-core input maps."""
    assert pair.shape == (O, L, L, DP) and bias.shape == (O, L, L, DP)
    assert np.abs(ln_pair_b).max() == 0 and np.abs(ln_bias_b).max() == 0, \
        "kernel folds LN beta=0; nonzero beta not implemented"
    f32 = np.float32
    bf16 = ml_dtypes.bfloat16
    shared = {
        "wqt": np.ascontiguousarray(Wq.T, f32),
        "wkt": np.ascontiguousarray(Wk.T, f32),
        "wvt": np.ascontiguousarray(Wv.T, f32),
        "wgt": np.ascontiguousarray(Wg.T, f32),
        "wot": np.ascontiguousarray(Wo.T, f32),
        "wbt": np.ascontiguousarray(Wb.T, f32),
        "vecs": np.ascontiguousarray(
            np.stack([ln_pair_g, ln_bias_g, bg, bo], axis=1), f32),
        "borow": np.ascontiguousarray(bo[None, :], f32),
    }
    pair_b = pair[0].astype(bf16)
    bias_b = bias[0].astype(bf16)
    in_maps = []
    for c in range(NCORES):
        S = slice(c * NS, (c + 1) * NS)
        m = dict(shared)
        m["pairc"] = np.ascontiguousarray(
            pair_b[:, S, :].transpose(1, 0, 2).reshape(R, DP))
        m["biasc"] = np.ascontiguousarray(
            bias_b[:, S, :].transpose(1, 0, 2).reshape(R, DP))
        in_maps.append(m)
    return in_maps


def gather_outputs(results):
    res = np.zeros((O, L, L, DP), np.float32)
    for c in range(NCORES):
        F = results[c]["outc"].astype(np.float32)       # [DP, R] transposed
        res[0, :, c * NS:(c + 1) * NS, :] = \
            F.T.reshape(NS, L, DP).transpose(1, 0, 2)
    return res


def kernel(**inputs):
    inputs = {k: np.asarray(v) for k, v in inputs.items()}
    nc = _build_program()
    in_maps = shard_inputs(**inputs)
    r = run_bass_kernel_spmd(nc, in_maps, core_ids=list(range(NCORES)))
    return gather_outputs(r.results)


def _ensure_ntff_hook():
    """The agent image's antenv lacks axon_hooks; recreate the registry and
    wire the ctypes NTFF hook from trn_agent_boot (profiling-only path)."""
    try:
        from antenv.axon_hooks import get_axon_ntff_profile_hook  # noqa: F401
        return
    except ImportError:
        pass
    import types
    import antenv
    mod = types.ModuleType("antenv.axon_hooks")
    mod._hook = None
    mod.set_axon_ntff_profile_hook = lambda h: setattr(mod, "_hook", h)
    mod.get_axon_ntff_profile_hook = lambda: mod._hook
    sys.modules["antenv.axon_hooks"] = mod
    antenv.axon_hooks = mod
    try:
        from trn_agent_boot.trn_boot import _ntff_profile_via_ctypes
        hook = _ntff_profile_via_ctypes("/opt/axon/libaxon_pjrt.so")
        if hook is not None:
            mod._hook = hook
    except Exception as e:  # profiling degrades, run still works
        print(f"NTFF hook setup failed: {e}", file=sys.stderr)


def kernel_profiled(**inputs):
    """Like kernel() but also returns exec-time info from neuron-profile."""
    inputs = {k: np.asarray(v) for k, v in inputs.items()}
    _ensure_ntff_hook()
    import concourse.bass_utils as bu
    bu.upload_artifacts = lambda tmpdir: f"local:{tmpdir}"  # no bucket here
    nc = _build_program()
    in_maps = shard_inputs(**inputs)
    r = run_bass_kernel_spmd(nc, in_maps, core_ids=list(range(NCORES)),
                             trace=True, trace_cores=list(range(NCORES)))
    return gather_outputs(r.results), r
